# revision 12
# baseline (speedup 1.0000x reference)
"""Bahdanau additive attention kernel for Trainium2 (8 NeuronCores).

score[b,t,s] = sum_e Wc[e] * tanh( (dec[b] @ Wd)[t,e] + (enc[b] @ We)[s,e] )
attn = softmax(score, axis=-1)

Sharding: data-parallel over batch B=8 -> one batch per core, weights
replicated. No collectives.

Per-core dataflow (all shapes [partition, free]):
  - PE-transpose decoder/encoder blocks (fp32, via identity matmul)
  - project:  oneT[e,t] = Wd^T dec^T   (e on partitions, 2 half-tiles)
              twoT[e,s] = We^T enc^T
  - for each t: DVE tensor_scalar_add broadcasts oneT[:,t] over twoT -> sum
  - ScalarE Tanh over big [128, Tc*Te] chunks (ACT is the bottleneck engine)
  - PE matvec: score[t,:] = Wc^T @ tanh_tile accumulated into PSUM rows
  - softmax epilogue: reduce_max(negate) -> Exp(bias=-max, accum_out=den)
                      -> reciprocal -> tensor_scalar_mul
"""

import numpy as np

B, Td, Te, D = 8, 128, 256, 256
P = 128  # partitions
EH = D // P  # 2 e-halves
T_C = 32  # decoder timesteps per tanh chunk
N_CORES = 8

# matvec strategy: "row_offset" = M=1 matmul writing psum row t directly;
# "shifted" = M=128 matmul with a shifted zero-padded weight column view.
MATVEC_MODE = "shifted"

_NC_CACHE = {}


def build_nc():
    from contextlib import ExitStack

    import concourse.bacc as bacc
    import concourse.bass as bass
    import concourse.mybir as mybir
    import concourse.tile as tile
    from concourse import masks

    f32 = mybir.dt.float32
    f32r = mybir.dt.float32r
    bf16 = mybir.dt.bfloat16

    nc = bacc.Bacc("TRN2", target_bir_lowering=False, debug=False)

    enc_d = nc.declare_dram_parameter("encoder_out_seq", [Te, D], f32, isOutput=False)
    dec_d = nc.declare_dram_parameter("decoder_out_seq", [Td, D], f32, isOutput=False)
    wd_d = nc.declare_dram_parameter("W_decoder", [D, D], f32, isOutput=False)
    we_d = nc.declare_dram_parameter("W_encoder", [D, D], f32, isOutput=False)
    wc_d = nc.declare_dram_parameter("W_combined", [D, 1], f32, isOutput=False)
    score_d = nc.declare_dram_parameter("score", [Td, Te], f32, isOutput=True)
    attn_d = nc.declare_dram_parameter("attn", [Td, Te], f32, isOutput=True)

    with tile.TileContext(nc) as tc, ExitStack() as ctx:
        consts = ctx.enter_context(tc.tile_pool(name="consts", bufs=1))
        psum_misc = ctx.enter_context(
            tc.tile_pool(name="psum_misc", bufs=2, space=bass.MemorySpace.PSUM)
        )
        psum_score = ctx.enter_context(
            tc.tile_pool(name="psum_score", bufs=1, space=bass.MemorySpace.PSUM)
        )
        sum_pool = ctx.enter_context(tc.tile_pool(name="sums", bufs=3))
        tanh_pool = ctx.enter_context(tc.tile_pool(name="tanhs", bufs=4))

        ident = consts.tile([P, P], f32)
        masks.make_identity(nc, ident[:])

        # ---- load inputs ----
        dec_sb = consts.tile([P, D], f32)  # [t, d]
        nc.sync.dma_start(dec_sb[:], dec_d[:])
        enc_sb = consts.tile([P, 2, D], f32)  # [s_p, s_half, d]
        for i in range(2):
            nc.sync.dma_start(enc_sb[:, i, :], enc_d[i * P : (i + 1) * P, :])
        wd_sb = consts.tile([P, EH, D], f32)  # [d_p, d_half, e]
        we_sb = consts.tile([P, EH, D], f32)
        for k in range(EH):
            nc.sync.dma_start(wd_sb[:, k, :], wd_d[k * P : (k + 1) * P, :])
            nc.sync.dma_start(we_sb[:, k, :], we_d[k * P : (k + 1) * P, :])
        w_sb = consts.tile([P, EH], f32)  # [e_p, e_half] combined weight
        for h in range(EH):
            nc.sync.dma_start(w_sb[:, h : h + 1], wc_d[h * P : (h + 1) * P, :])

        if MATVEC_MODE == "shifted":
            # z_sb[h] is [P, 2*P] with column P holding w-half h, zeros else.
            # Produced by DVE only, so matvec LDWEIGHTS needs just one wait.
            z_sb = consts.tile([P, EH, 2 * P], bf16)
            nc.vector.memset(z_sb[:], 0.0)
            for h in range(EH):
                nc.vector.tensor_copy(z_sb[:, h, P : P + 1], w_sb[:, h : h + 1])

        # ---- transposes (PE) ----
        decT_sb = consts.tile([P, EH, Td], f32)  # [d_p, d_half, t]
        for k in range(EH):
            pt = psum_misc.tile([P, P], f32, tag="pt")
            nc.tensor.transpose(pt[:], dec_sb[:, k * P : (k + 1) * P], ident[:])
            nc.vector.tensor_copy(decT_sb[:, k, :], pt[:])
        encT_sb = consts.tile([P, EH, Te], f32)  # [d_p, d_half, s]
        for k in range(EH):
            for i in range(2):
                pt = psum_misc.tile([P, P], f32, tag="pt")
                nc.tensor.transpose(
                    pt[:], enc_sb[:, i, k * P : (k + 1) * P], ident[:]
                )
                nc.vector.tensor_copy(encT_sb[:, k, i * P : (i + 1) * P], pt[:])

        # ---- projections ----
        oneT_sb = consts.tile([P, EH, Td], f32)  # [e_p, e_half, t]
        twoT_sb = consts.tile([P, EH, Te], bf16)  # [e_p, e_half, s]
        for h in range(EH):
            pp_full = psum_misc.tile([P, Te], f32, tag="proj")
            pp = pp_full[:, :Td]
            for k in range(EH):
                nc.tensor.matmul(
                    pp[:],
                    wd_sb[:, k, h * P : (h + 1) * P],
                    decT_sb[:, k, :],
                    start=(k == 0),
                    stop=(k == EH - 1),
                )
            nc.vector.tensor_copy(oneT_sb[:, h, :], pp[:])
            pq = psum_misc.tile([P, Te], f32, tag="proj")
            for k in range(EH):
                nc.tensor.matmul(
                    pq[:],
                    we_sb[:, k, h * P : (h + 1) * P],
                    encT_sb[:, k, :],
                    start=(k == 0),
                    stop=(k == EH - 1),
                )
            nc.vector.tensor_copy(twoT_sb[:, h, :], pq[:])

        # ---- main: broadcast-add, tanh, matvec ----
        score_ps = psum_score.tile([P, Te], f32)
        n_chunks = Td // T_C
        for c in range(n_chunks):
            t0 = c * T_C
            tanh_tiles = []
            for h in range(EH):
                sum_t = sum_pool.tile([P, T_C, Te], bf16, tag="sum")
                for j in range(T_C):
                    t = t0 + j
                    nc.vector.tensor_scalar_add(
                        sum_t[:, j, :], twoT_sb[:, h, :], oneT_sb[:, h, t : t + 1]
                    )
                tanh_t = tanh_pool.tile([P, T_C, Te], bf16, tag="tanh")
                nc.scalar.activation(
                    tanh_t[:], sum_t[:], mybir.ActivationFunctionType.Tanh
                )
                tanh_tiles.append(tanh_t)

            for j in range(T_C):
                t = t0 + j
                for h in range(EH):
                    if MATVEC_MODE == "row_offset":
                        nc.tensor.matmul(
                            score_ps[t : t + 1, :],
                            w_sb[:, h : h + 1],
                            tanh_tiles[h][:, j, :],
                            start=(h == 0),
                            stop=(h == EH - 1),
                        )
                    else:
                        nc.tensor.matmul(
                            score_ps[:],
                            z_sb[:, h, P - t : 2 * P - t],
                            tanh_tiles[h][:, j, :],
                            start=(c == 0 and j == 0 and h == 0),
                            stop=(c == n_chunks - 1 and j == T_C - 1 and h == EH - 1),
                            skip_group_check=True,
                        )

        # ---- softmax epilogue ----
        score_sb = consts.tile([P, Te], f32)
        nc.vector.tensor_copy(score_sb[:], score_ps[:])
        nc.sync.dma_start(score_d[:], score_sb[:])

        negmax = consts.tile([P, 1], f32)
        nc.vector.reduce_max(
            negmax[:], score_sb[:], axis=mybir.AxisListType.X, negate=True
        )
        exp_sb = consts.tile([P, Te], f32)
        denom = consts.tile([P, 1], f32)
        nc.scalar.activation(
            exp_sb[:],
            score_sb[:],
            mybir.ActivationFunctionType.Exp,
            bias=negmax[:],
            accum_out=denom[:],
        )
        recip = consts.tile([P, 1], f32)
        nc.vector.reciprocal(recip[:], denom[:])
        attn_sb = consts.tile([P, Te], f32)
        nc.vector.tensor_scalar_mul(attn_sb[:], exp_sb[:], recip[:])
        nc.sync.dma_start(attn_d[:], attn_sb[:])

    nc.compile()
    return nc


def get_nc():
    if "nc" not in _NC_CACHE:
        _NC_CACHE["nc"] = build_nc()
    return _NC_CACHE["nc"]


def kernel(**inputs) -> np.ndarray:
    from concourse.bass_utils import run_bass_kernel_spmd

    enc = np.ascontiguousarray(np.asarray(inputs["encoder_out_seq"], np.float32))
    dec = np.ascontiguousarray(np.asarray(inputs["decoder_out_seq"], np.float32))
    wd = np.ascontiguousarray(np.asarray(inputs["W_decoder"], np.float32))
    we = np.ascontiguousarray(np.asarray(inputs["W_encoder"], np.float32))
    wc = np.ascontiguousarray(np.asarray(inputs["W_combined"], np.float32))

    nc = get_nc()
    in_maps = [
        {
            "encoder_out_seq": enc[b],
            "decoder_out_seq": dec[b],
            "W_decoder": wd,
            "W_encoder": we,
            "W_combined": wc,
        }
        for b in range(N_CORES)
    ]
    res = run_bass_kernel_spmd(nc, in_maps, core_ids=list(range(N_CORES)))
    score = np.stack([res.results[b]["score"] for b in range(N_CORES)])
    attn = np.stack([res.results[b]["attn"] for b in range(N_CORES)])
    return score, attn


# revision 15
# speedup vs baseline: 1.0044x; 1.0044x over previous
"""Bahdanau additive attention kernel for Trainium2 (8 NeuronCores).

score[b,t,s] = sum_e Wc[e] * tanh( (dec[b] @ Wd)[t,e] + (enc[b] @ We)[s,e] )
attn = softmax(score, axis=-1)

Sharding: data-parallel over batch B=8 -> one batch per core, weights
replicated. No collectives.

Per-core dataflow (all shapes [partition, free]):
  - PE-transpose decoder/encoder blocks (fp32, via identity matmul)
  - project:  oneT[e,t] = Wd^T dec^T   (e on partitions, 2 half-tiles)
              twoT[e,s] = We^T enc^T   (bf16 out)
  - for each t: DVE tensor_scalar_add broadcasts oneT[:,t] over twoT -> sum
  - ScalarE Tanh over big [128, Tc*Te] chunks (ACT is the bottleneck engine)
  - PE matvec, pair-packed: one matmul per (t-pair, e-half) with a
    shifted zero-padded weight column (M=64, N=512) accumulates
    psum row i = [score_{2i} | score_{2i+1}] for i in 0..63
  - softmax on the packed [64, 2, 256] layout (segment-wise), pair rows
    DMA out contiguously.
"""

import numpy as np

B, Td, Te, D = 8, 128, 256, 256
P = 128  # partitions
EH = D // P  # 2 e-halves
T_C = 32  # decoder timesteps per tanh chunk
NPAIR = Td // 2  # 64 pair-rows
N_CORES = 8

_NC_CACHE = {}


def build_nc():
    from contextlib import ExitStack

    import concourse.bacc as bacc
    import concourse.bass as bass
    import concourse.mybir as mybir
    import concourse.tile as tile
    from concourse import masks

    f32 = mybir.dt.float32
    bf16 = mybir.dt.bfloat16
    AF = mybir.ActivationFunctionType

    nc = bacc.Bacc("TRN2", target_bir_lowering=False, debug=False)

    enc_d = nc.declare_dram_parameter("encoder_out_seq", [Te, D], f32, isOutput=False)
    dec_d = nc.declare_dram_parameter("decoder_out_seq", [Td, D], f32, isOutput=False)
    wd_d = nc.declare_dram_parameter("W_decoder", [D, D], f32, isOutput=False)
    we_d = nc.declare_dram_parameter("W_encoder", [D, D], f32, isOutput=False)
    wc_d = nc.declare_dram_parameter("W_combined", [D, 1], f32, isOutput=False)
    score_d = nc.declare_dram_parameter("score", [Td, Te], f32, isOutput=True)
    attn_d = nc.declare_dram_parameter("attn", [Td, Te], f32, isOutput=True)

    # pair-row views of the outputs: sbuf row i -> DRAM rows (2i, 2i+1)
    score_pairs = score_d[:].rearrange("(i two) s -> i two s", two=2)
    attn_pairs = attn_d[:].rearrange("(i two) s -> i two s", two=2)

    with tile.TileContext(nc) as tc, ExitStack() as ctx:
        consts = ctx.enter_context(tc.tile_pool(name="consts", bufs=1))
        psum_misc = ctx.enter_context(
            tc.tile_pool(name="psum_misc", bufs=2, space=bass.MemorySpace.PSUM)
        )
        psum_score = ctx.enter_context(
            tc.tile_pool(name="psum_score", bufs=1, space=bass.MemorySpace.PSUM)
        )
        sum_pool = ctx.enter_context(tc.tile_pool(name="sums", bufs=3))
        tanh_pool = ctx.enter_context(tc.tile_pool(name="tanhs", bufs=4))

        ident = consts.tile([P, P], f32)
        masks.make_identity(nc, ident[:])

        # ---- load inputs ----
        dec_sb = consts.tile([P, D], f32)  # [t, d]
        nc.sync.dma_start(dec_sb[:], dec_d[:])
        enc_sb = consts.tile([P, 2, D], f32)  # [s_p, s_half, d]
        for i in range(2):
            nc.sync.dma_start(enc_sb[:, i, :], enc_d[i * P : (i + 1) * P, :])
        wd_sb = consts.tile([P, EH, D], f32)  # [d_p, d_half, e]
        we_sb = consts.tile([P, EH, D], f32)
        for k in range(EH):
            nc.sync.dma_start(wd_sb[:, k, :], wd_d[k * P : (k + 1) * P, :])
            nc.sync.dma_start(we_sb[:, k, :], we_d[k * P : (k + 1) * P, :])
        w_sb = consts.tile([P, EH], f32)  # [e_p, e_half] combined weight
        for h in range(EH):
            nc.sync.dma_start(w_sb[:, h : h + 1], wc_d[h * P : (h + 1) * P, :])

        # z2_sb[h]: [P, 2*NPAIR] bf16 with column NPAIR holding w-half h,
        # zeros elsewhere.  lhsT view z2_sb[:, h, NPAIR-i : 2*NPAIR-i] puts
        # w in column i -> matvec result lands in psum row i only.
        # Produced by DVE only, so matvec LDWEIGHTS needs few fresh waits.
        z2_sb = consts.tile([P, EH, 2 * NPAIR], bf16)
        nc.vector.memset(z2_sb[:], 0.0)
        for h in range(EH):
            nc.vector.tensor_copy(z2_sb[:, h, NPAIR : NPAIR + 1], w_sb[:, h : h + 1])

        # ---- transposes (PE) ----
        decT_sb = consts.tile([P, EH, Td], f32)  # [d_p, d_half, t]
        for k in range(EH):
            pt = psum_misc.tile([P, P], f32, tag="pt")
            nc.tensor.transpose(pt[:], dec_sb[:, k * P : (k + 1) * P], ident[:])
            nc.vector.tensor_copy(decT_sb[:, k, :], pt[:])
        encT_sb = consts.tile([P, EH, Te], f32)  # [d_p, d_half, s]
        for k in range(EH):
            for i in range(2):
                pt = psum_misc.tile([P, P], f32, tag="pt")
                nc.tensor.transpose(pt[:], enc_sb[:, i, k * P : (k + 1) * P], ident[:])
                nc.vector.tensor_copy(encT_sb[:, k, i * P : (i + 1) * P], pt[:])

        # ---- projections (PSUM fp32 -> SBUF bf16) ----
        oneT_sb = consts.tile([P, EH, Td], f32)  # [e_p, e_half, t]
        twoT_sb = consts.tile([P, EH, Te], bf16)  # [e_p, e_half, s]
        for h in range(EH):
            pp_full = psum_misc.tile([P, Te], f32, tag="proj")
            pp = pp_full[:, :Td]
            for k in range(EH):
                nc.tensor.matmul(
                    pp[:],
                    wd_sb[:, k, h * P : (h + 1) * P],
                    decT_sb[:, k, :],
                    start=(k == 0),
                    stop=(k == EH - 1),
                )
            nc.vector.tensor_copy(oneT_sb[:, h, :], pp[:])
            pq = psum_misc.tile([P, Te], f32, tag="proj")
            for k in range(EH):
                nc.tensor.matmul(
                    pq[:],
                    we_sb[:, k, h * P : (h + 1) * P],
                    encT_sb[:, k, :],
                    start=(k == 0),
                    stop=(k == EH - 1),
                )
            nc.vector.tensor_copy(twoT_sb[:, h, :], pq[:])

        # ---- main: broadcast-add, tanh, pair-packed matvec ----
        score_ps = psum_score.tile([NPAIR, 2 * Te], f32)  # row i = [t=2i | t=2i+1]
        n_chunks = Td // T_C
        for c in range(n_chunks):
            t0 = c * T_C
            tanh_tiles = []
            for h in range(EH):
                sum_t = sum_pool.tile([P, T_C, Te], bf16, tag="sum")
                for j in range(T_C):
                    t = t0 + j
                    nc.vector.tensor_scalar_add(
                        sum_t[:, j, :], twoT_sb[:, h, :], oneT_sb[:, h, t : t + 1]
                    )
                tanh_t = tanh_pool.tile([P, T_C, Te], bf16, tag="tanh")
                nc.scalar.activation(tanh_t[:], sum_t[:], AF.Tanh)
                tanh_tiles.append(tanh_t)

            for j in range(0, T_C, 2):
                i = (t0 + j) // 2  # pair-row index
                for h in range(EH):
                    nc.tensor.matmul(
                        score_ps[:, :],
                        z2_sb[:, h, NPAIR - i : 2 * NPAIR - i],
                        tanh_tiles[h][:, j : j + 2, :],
                        start=(c == 0 and j == 0 and h == 0),
                        stop=(c == n_chunks - 1 and j == T_C - 2 and h == EH - 1),
                        skip_group_check=True,
                    )

        # ---- softmax epilogue on packed [64, 2*256] layout ----
        # All segment ops use standard tensor_scalar / activation-bias forms.
        score_sb = consts.tile([NPAIR, 2, Te], f32)
        nc.vector.tensor_copy(score_sb[:], score_ps[:])
        negmax = consts.tile([NPAIR, 2], f32)
        nc.vector.reduce_max(
            negmax[:], score_sb[:], axis=mybir.AxisListType.X, negate=True
        )
        exp_sb = consts.tile([NPAIR, 2, Te], f32)
        den2 = consts.tile([NPAIR, 2], f32)
        recip = consts.tile([NPAIR, 2], f32)
        attn_sb = consts.tile([NPAIR, 2, Te], f32)
        for seg in range(2):
            nc.scalar.activation(
                exp_sb[:, seg, :],
                score_sb[:, seg, :],
                AF.Exp,
                bias=negmax[:, seg : seg + 1],
                accum_out=den2[:, seg : seg + 1],
            )
        nc.vector.reciprocal(recip[:], den2[:])
        for seg in range(2):
            nc.vector.tensor_scalar_mul(
                attn_sb[:, seg, :], exp_sb[:, seg, :], recip[:, seg : seg + 1]
            )
        # outputs: segment seg of sbuf row i -> DRAM row 2i+seg (stride-2 rows)
        for seg in range(2):
            nc.sync.dma_start(score_pairs[:, seg, :], score_sb[:, seg, :])
            nc.sync.dma_start(attn_pairs[:, seg, :], attn_sb[:, seg, :])

    nc.compile()
    return nc


def get_nc():
    if "nc" not in _NC_CACHE:
        _NC_CACHE["nc"] = build_nc()
    return _NC_CACHE["nc"]


def kernel(**inputs) -> np.ndarray:
    from concourse.bass_utils import run_bass_kernel_spmd

    enc = np.ascontiguousarray(np.asarray(inputs["encoder_out_seq"], np.float32))
    dec = np.ascontiguousarray(np.asarray(inputs["decoder_out_seq"], np.float32))
    wd = np.ascontiguousarray(np.asarray(inputs["W_decoder"], np.float32))
    we = np.ascontiguousarray(np.asarray(inputs["W_encoder"], np.float32))
    wc = np.ascontiguousarray(np.asarray(inputs["W_combined"], np.float32))

    nc = get_nc()
    in_maps = [
        {
            "encoder_out_seq": enc[b],
            "decoder_out_seq": dec[b],
            "W_decoder": wd,
            "W_encoder": we,
            "W_combined": wc,
        }
        for b in range(N_CORES)
    ]
    res = run_bass_kernel_spmd(nc, in_maps, core_ids=list(range(N_CORES)))
    score = np.stack([res.results[b]["score"] for b in range(N_CORES)])
    attn = np.stack([res.results[b]["attn"] for b in range(N_CORES)])
    return score, attn
